# revision 87
# baseline (speedup 1.0000x reference)
"""Trainium2 Bass kernel for a single transformer encoder layer with
Music-Transformer relative position attention (causal).

Sharding over 8 NeuronCores:
  - Attention: data-parallel over batch (2) x tensor-parallel over head
    pairs (4) -> core c handles batch c//4, heads {2g, 2g+1}, g = c%4.
  - ctx column-slices are AllGather'd (f16) within each 4-core group.
  - LayerNorm + FFN: row-parallel, core c handles rows [512g, 512g+512)
    of its batch; output assembled on host.

Attention inner loop (per 512-row superblock J, head hp):
  - raw rel-position strips W[q, m] = q . Er[e0_I + m] are computed by PE
    and copied (Pool engine) to SBUF f16 -- no exp on the strip.
  - the Music-Transformer skew is a 3-dim strided SBUF->SBUF DMA per
    512-key chunk reading all 4 row-blocks at once (m = 127+512t+kk-ql).
  - Srel is merged into the QK PSUM accumulation with an identity-
    stationary matmul (PE), so softmax needs a single Exp (Act) per
    chunk: exp(QK + Srel) directly, halving Act + freeing DVE.
  - causal mask: strip pads are finite, the diagonal 128-block of the
    exp'd scores is zeroed by a lower-tri f16 multiply (DVE).
  - A^T via PE transposes (4 per 512 chunk) + one DVE copy, then 65-wide
    AV matmuls with a ones-column denominator trick (as before).
Most tensors are f16 (weights, activations feeding matmuls); PSUM
accumulation and LayerNorm stay f32.
"""

import numpy as np

import concourse.bass as bass
import concourse.mybir as mybir
import concourse.tile as tile
from concourse import bacc
from concourse.bass import ts
from concourse.bass_utils import run_bass_kernel_spmd
from concourse.masks import make_identity

B, S, D, H, DH, FFN = 2, 2048, 512, 8, 64, 2048
EPS = 1e-5
NCORES = 8
GROUPS = [[0, 1, 2, 3], [4, 5, 6, 7]]
P = 128          # partitions
KB = D // P      # 4 contraction blocks for d_model
NI = S // P      # 16 row blocks
RT = 4           # row tiles per core in FFN phase (512 rows)
NF = FFN // P    # 16 ffn blocks
ERT_W = S + 512  # ErT padded with zeros so strip windows stay in range

f32 = mybir.dt.float32
f32r = mybir.dt.float32r
f16 = mybir.dt.float16
f8 = mybir.dt.float8e4

_COMPILED = {}


def build_nc(with_collective=True, phases=(0, 1, 2, 3)):
    nc = bacc.Bacc(None, num_devices=NCORES)

    # ---- per-core DRAM inputs (host pre-sliced / pre-transposed) ----
    xT = nc.dram_tensor("xT", [D, S], f16, kind="ExternalInput")        # x[b].T
    wq = nc.dram_tensor("wq", [D, P], f16, kind="ExternalInput")        # /8 folded
    wk = nc.dram_tensor("wk", [D, P], f16, kind="ExternalInput")
    wv = nc.dram_tensor("wv", [D, P], f16, kind="ExternalInput")
    bqkv = nc.dram_tensor("bqkv", [3, P], f32, kind="ExternalInput")    # bq/8, bk, bv
    ert = nc.dram_tensor("ert", [DH, ERT_W], f16, kind="ExternalInput") # Er.T zero-pad
    xres = nc.dram_tensor("xres", [512, D], f16, kind="ExternalInput")  # row slice
    w1 = nc.dram_tensor("w1", [D, FFN], f8, kind="ExternalInput")   # x64
    w2 = nc.dram_tensor("w2", [FFN, D], f8, kind="ExternalInput")   # x64
    b1 = nc.dram_tensor("b1", [P, NF], f32, kind="ExternalInput")       # transposed
    lnp = nc.dram_tensor("lnp", [5, D], f16, kind="ExternalInput")      # g1,be1,g2,be2,b2
    y = nc.dram_tensor("y", [512, D], f32, kind="ExternalOutput")

    with tile.TileContext(nc) as tc:
        with tc.tile_pool(name="persist", bufs=1) as pp, \
             tc.tile_pool(name="dram", bufs=1, space="DRAM") as dp:

            ccin = dp.tile([S, P], f16)
            # gathered ctx, split in three row-parts so the AllGathers fire
            # mid-attention and most FFN work hides under late attention
            ccout = [dp.tile([4, rn, P], f16, name=f"ccout{h}")
                     for h, rn in enumerate((1024, 512, 512))]

            qT = pp.tile([P, S], f16)      # 2 heads stacked on partitions
            kT = pp.tile([P, S], f16)
            # v natural + a ones column per head (row-sum trick):
            # cols [66h:66h+64]=v_h, col 66h+64 = 1.0, 66h+65 pad
            vv = pp.tile([P, NI, 132], f16)
            ident16 = pp.tile([P, P], f16)
            make_identity(nc, ident16)
            # lower-triangular (incl diagonal) fp16 mask for the causal
            # diagonal block: 1.0 where key <= query, else 0.0
            tri16 = pp.tile([P, P], f16)
            nc.gpsimd.memset(tri16, 1.0)
            nc.gpsimd.affine_select(
                out=tri16, in_=tri16, base=0, channel_multiplier=1,
                pattern=[[-1, P]], compare_op=mybir.AluOpType.is_ge,
                fill=0.0)
            # ErT replicated in both partition halves so it can pair with
            # either head's qT slice (matmul requires equal base partitions);
            # DMAs issued after the projection inputs (not needed until
            # the strip matmuls)
            ert_sb = pp.tile([P, ERT_W], f16)
            # FFN weights prefetched into a long-lived pool so their DMAs
            # overlap the attention phase (emission deferred past phase 0 so
            # they queue behind the xT/projection loads)
            w1_sb = pp.tile([P, KB, FFN], f8)
            w2_sb = pp.tile([P, NF, D], f8)

            # ---------------- Phase 0: projections ----------------
            with tc.tile_pool(name="p0", bufs=1) as p0, \
                 tc.tile_pool(name="p0ps", bufs=2, space="PSUM") as p0ps:
                xT_sb = p0.tile([P, KB, S], f16)
                xT_r = xT.rearrange("(kk p) s -> p kk s", p=P)
                w_sb = {}
                for nm, t in (("q", wq), ("k", wk), ("v", wv)):
                    w_sb[nm] = p0.tile([P, KB, P], f16, tag=f"w{nm}",
                                       name=f"w{nm}_sb")
                    nc.sync.dma_start(out=w_sb[nm],
                                      in_=t.rearrange("(kk p) m -> p kk m", p=P))
                nc.sync.dma_start(out=xT_sb[:, :, ts(0, 512)],
                                  in_=xT_r[:, :, ts(0, 512)])
                vT16 = p0.tile([P, S], f16)
                # biases: one contiguous DMA + PE transpose to partition-major
                # (per-column DMAs would be 128 four-byte descriptors each)
                brow = p0.tile([3, P], f32)
                nc.sync.dma_start(out=brow, in_=bqkv[:])
                for n in range(1, S // 512):
                    nc.sync.dma_start(out=xT_sb[:, :, ts(n, 512)],
                                      in_=xT_r[:, :, ts(n, 512)])
                nc.sync.dma_start(out=ert_sb[0:DH, :], in_=ert[:])
                nc.sync.dma_start(out=ert_sb[DH:P, :], in_=ert[:])
                ident32a = p0.tile([P, P], f32)
                make_identity(nc, ident32a)
                btp = p0ps.tile([P, 3], f32, tag="btp")
                nc.tensor.matmul(btp, brow, ident32a[0:3, 0:3],
                                 is_transpose=True)
                btile = p0.tile([P, 3], f32)
                nc.vector.tensor_copy(out=btile, in_=btp)
                for n in range(S // 512):
                    for idx, (nm, dst) in enumerate(
                            (("q", qT), ("k", kT), ("v", None))):
                        ps = p0ps.tile([P, 512], f32, tag="pp", bufs=4,
                                       name="ps")
                        for kk in range(KB):
                            nc.tensor.matmul(ps, w_sb[nm][:, kk, :],
                                             xT_sb[:, kk, ts(n, 512)],
                                             start=(kk == 0), stop=(kk == KB - 1))
                        # bias add on the Act engine (idle during P0):
                        # out = Copy(in * 1.0 + bias)
                        tgt = vT16 if nm == "v" else dst
                        nc.scalar.activation(
                            out=tgt[:, ts(n, 512)], in_=ps,
                            func=mybir.ActivationFunctionType.Identity,
                            bias=btile[:, idx:idx + 1], scale=1.0)
                # v natural via PE transpose of vT16
                nc.vector.memset(vv[:, :, 64:65], 1.0)
                nc.vector.memset(vv[:, :, 130:131], 1.0)
                for t in range(NI):
                    trp = p0ps.tile([P, P], f16, tag="ptr", bufs=2)
                    nc.tensor.transpose(trp, vT16[:, ts(t, P)], ident16)
                    nc.vector.tensor_copy(out=vv[:, t, 0:DH],
                                          in_=trp[:, 0:DH])
                    nc.vector.tensor_copy(out=vv[:, t, 66:66 + DH],
                                          in_=trp[:, DH:P])

            # ---------------- Phase 1: attention ----------------
            # P3 preamble tiles live in the persist pool and load during
            # attention so the post-collective critical path is short
            lnp_sb = pp.tile([P, 5, D], f16)
            b1_sb = pp.tile([P, NF], f32)
            xr_sb = pp.tile([P, RT, D], f16)
            eps_sb = pp.tile([P, 1], f32)
            ident32 = pp.tile([P, P], f32)
            if 3 in phases:
                nc.sync.dma_start(out=w1_sb,
                                  in_=w1.rearrange("(kk p) n -> p kk n", p=P))
                nc.sync.dma_start(out=w2_sb,
                                  in_=w2.rearrange("(ff p) n -> p ff n", p=P))
                nc.sync.dma_start(
                    out=lnp_sb,
                    in_=bass.AP(tensor=lnp[:].tensor, offset=0,
                                ap=[[0, P], [D, 5], [1, D]]))
                nc.sync.dma_start(out=b1_sb, in_=b1[:])
                nc.sync.dma_start(out=xr_sb,
                                  in_=xres.rearrange("(t p) d -> p t d", p=P))
                nc.vector.memset(eps_sb, EPS)
                make_identity(nc, ident32)
            with tc.tile_pool(name="p1", bufs=2) as p1, \
                 tc.tile_pool(name="p1s", bufs=4) as p1s, \
                 tc.tile_pool(name="p1ps", bufs=2, space="PSUM") as p1ps, \
                 tc.tile_pool(name="p1px", bufs=1, space="PSUM") as p1px:
                WMX = 512 * 4 + 128  # one strip shape for all J

                def strip_copy(isub, dst, src):
                    # fan the PSUM->SBUF strip copies over two engines so
                    # neither paces the gather chain (GPSIMD can't touch PSUM)
                    if isub % 2 == 0:
                        nc.scalar.copy(out=dst, in_=src)
                    else:
                        nc.vector.tensor_copy(out=dst, in_=src)

                esrs = {}

                def stage_strips(J, hp):
                    # strips (raw, f16) + skew gathers for one (J, head);
                    # generator: yields after each 512-col chunk group so the
                    # driver can interleave with score processing
                    WJ = 512 * (J + 1)
                    h0 = DH * hp
                    strip = p1.tile([P, 4, WMX], f16, tag="st",
                                    name=f"strip{J}_{hp}", bufs=3)
                    # pad: finite values for the (masked) diagonal
                    # overshoot region so exp() can't see NaN garbage;
                    # the gather reads at most 126 elems past WJ
                    nc.vector.memset(strip[:, :, WJ:WJ + P], 0.0)
                    esrs[(J, hp)] = []

                    def gather(t0):
                        # skewed read: dest (jl, isub, kk) pulls
                        # strip[jl + ..., isub, 127 + 512*t0 + kk - ql]
                        skew_ap = bass.AP(
                            tensor=strip.tensor,
                            offset=strip.offset + 127 + 512 * t0,
                            ap=[[4 * WMX - 1, P], [WMX, 4], [1, 512]])
                        esr = p1s.tile([P, 4, 512], f16, tag="esr",
                                       name=f"esr{J}_{hp}_{t0}", bufs=10)
                        with tc.high_priority(offset=600):
                            nc.sync.dma_start(out=esr, in_=skew_ap)
                        esrs[(J, hp)].append(esr)

                    for c in range(J + 1):
                        for isub in range(4):
                            I = 4 * J + isub
                            e0 = S - P * (I + 1)
                            with tc.high_priority(offset=700):
                                sps = p1ps.tile([P, 512], f32, tag="mm",
                                                name="sps", bufs=4)
                                nc.tensor.matmul(
                                    sps, qT[h0:h0 + DH, ts(I, P)],
                                    ert_sb[h0:h0 + DH,
                                           e0 + 512 * c:e0 + 512 * (c + 1)],
                                    start=True, stop=True)
                                strip_copy(isub,
                                           strip[:, isub, ts(c, 512)], sps)
                        if c >= 1:
                            gather(c - 1)
                        yield
                    gather(J)

                def stage_proc(J, hp):
                    # scores + softmax + AV for one (J, head); generator:
                    # yields after each row-block so strip chunks of a later
                    # stage can be interleaved without head-blocking PE
                    h0 = DH * hp
                    for isub in range(4):
                            I = 4 * J + isub
                            LI = P * (I + 1)
                            nch = (LI + 511) // 512
                            eqk = p1.tile([P, S], f16, tag="eqk",
                                          name=f"eqk{J}_{hp}_{isub}", bufs=3)
                            for c in range(nch):
                                m0 = 512 * c
                                ml = min(512, LI - m0)
                                qk = p1ps.tile([P, 512], f32, tag="mm",
                                               name="qk", bufs=4)
                                nc.tensor.matmul(qk[:, :ml],
                                                 qT[h0:h0 + DH, ts(I, P)],
                                                 kT[h0:h0 + DH, m0:m0 + ml],
                                                 start=True, stop=False)
                                # Srel merge: identity-stationary matmul
                                # accumulates the skewed strip into PSUM
                                nc.tensor.matmul(qk[:, :ml], ident16,
                                                 esrs[(J, hp)][c][:, isub, :ml],
                                                 start=False, stop=True)
                                nc.scalar.activation(
                                    out=eqk[:, m0:m0 + ml], in_=qk[:, :ml],
                                    func=mybir.ActivationFunctionType.Exp)
                            # zero future keys in the causal diagonal block
                            nc.gpsimd.tensor_tensor(
                                out=eqk[:, LI - P:LI], in0=eqk[:, LI - P:LI],
                                in1=tri16, op=mybir.AluOpType.mult)
                            pctx = p1px.tile([P, 65], f32, tag="pctx",
                                             name="pctx", bufs=2)
                            blk = 0
                            for c in range(nch):
                                m0 = 512 * c
                                ml = min(512, LI - m0)
                                nsub = ml // P
                                ptr4 = p1ps.tile([P, 512], f16, tag="ptr4",
                                                 name="ptr4", bufs=2)
                                for j in range(nsub):
                                    nc.tensor.transpose(
                                        ptr4[:, ts(j, P)],
                                        eqk[:, m0 + P * j:m0 + P * (j + 1)],
                                        ident16)
                                aT4 = p1s.tile([P, 512], f16, tag="aT4",
                                               name="aT4", bufs=4)
                                nc.vector.tensor_copy(out=aT4[:, :ml],
                                                      in_=ptr4[:, :ml])
                                for j in range(nsub):
                                    t = (m0 + P * j) // P
                                    nc.tensor.matmul(
                                        pctx, aT4[:, ts(j, P)],
                                        vv[:, t, 66 * hp:66 * hp + 65],
                                        start=(blk == 0),
                                        stop=(blk == I))
                                    blk += 1
                            denom = p1s.tile([P, 1], f32, tag="dn",
                                             name="dn", bufs=2)
                            nc.vector.reciprocal(out=denom, in_=pctx[:, 64:65])
                            ctxs = p1s.tile([P, DH], f16, tag="cx",
                                            name="cx", bufs=2)
                            nc.vector.tensor_scalar_mul(out=ctxs,
                                                        in0=pctx[:, 0:DH],
                                                        scalar1=denom)
                            nc.sync.dma_start(
                                out=ccin[ts(I, P), h0:h0 + DH], in_=ctxs)
                            yield

                def layer_norm(dst, src, gamma_i, beta_i, tagp, fast=False):
                    # fast=True keeps the gamma/beta ops on DVE -- used for
                    # the final row-tile whose chain is the kernel tail
                    eng = nc.vector if fast else nc.gpsimd
                    stats = p1s.tile([P, 6], f32, tag=f"st{tagp}")
                    mv = p1s.tile([P, 2], f32, tag=f"mv{tagp}")
                    nc.vector.bn_stats(out=stats, in_=src)
                    nc.vector.bn_aggr(out=mv, in_=stats)
                    rstd = p1s.tile([P, 1], f32, tag=f"rs{tagp}")
                    nc.scalar.activation(out=rstd, in_=mv[:, 1:2],
                                         func=mybir.ActivationFunctionType.Sqrt,
                                         bias=eps_sb, scale=1.0)
                    nc.vector.reciprocal(out=rstd, in_=rstd)
                    nc.vector.tensor_scalar(out=dst, in0=src,
                                            scalar1=mv[:, 0:1], scalar2=rstd,
                                            op0=mybir.AluOpType.subtract,
                                            op1=mybir.AluOpType.mult)
                    eng.tensor_tensor(out=dst, in0=dst,
                                      in1=lnp_sb[:, gamma_i, :],
                                      op=mybir.AluOpType.mult)
                    eng.tensor_tensor(out=dst, in0=dst,
                                      in1=lnp_sb[:, beta_i, :],
                                      op=mybir.AluOpType.add)

                # ctx gathered in three parts: rows [0, 1024) after the
                # J=1 stages, [1024, 1536) after J=2, [1536, 2048) after
                # J=3 -- so only the last 128-row tile's FFN is exposed
                # in the tail after attention ends
                PARTS = [(0, 1024, 2), (1024, 512, 1), (1536, 512, 1)]

                def collective_part(p):
                    r0, rn, _ = PARTS[p]
                    cslice = ccin[r0:r0 + rn, :]
                    if with_collective:
                        nc.gpsimd.collective_compute(
                            "AllGather", mybir.AluOpType.bypass,
                            replica_groups=GROUPS,
                            ins=[cslice.opt()], outs=[ccout[p][:].opt()])
                    else:  # timeline-sim variant: local copy stands in
                        nc.sync.dma_start(out=ccout[p][0], in_=cslice)

                def phase3_part(p, rsnap):
                    # LN1 + FFN + LN2 over this core's share of gathered
                    # part p (nt row-tiles of 128)
                    r0, rn, nt = PARTS[p]
                    toff = sum(q[2] for q in PARTS[:p])  # xr/y tile offset
                    sz = f"{nt}"
                    h16 = p1.tile([P, nt, D], f16, tag=f"h16{sz}", bufs=1,
                                  name=f"h16_{p}")
                    for t in range(nt):
                        nc.sync.dma_start(
                            out=h16[:, t, :],
                            in_=ccout[p][:, bass.ds(rsnap + t * P, P), :]
                            .rearrange("q p c -> p q c"))
                    h_sb = p1.tile([P, nt, D], f16, tag=f"hsb{sz}", bufs=1,
                                   name=f"hsb_{p}")
                    h1 = p1.tile([P, nt, D], f32, tag=f"h1{sz}", bufs=1,
                                 name=f"h1_{p}")
                    h1T = p1.tile([P, KB, nt * P], f8, tag=f"h1T{sz}",
                                  bufs=1, name=f"h1T_{p}")
                    gT = p1.tile([P, NF, nt * P], f8, tag=f"gT{sz}", bufs=1,
                                 name=f"gT_{p}")
                    for t in range(nt):
                        eng1 = nc.vector if p == 2 else nc.gpsimd
                        eng1.tensor_tensor(out=h_sb[:, t, :],
                                           in0=h16[:, t, :],
                                           in1=xr_sb[:, toff + t, :],
                                           op=mybir.AluOpType.add)
                        layer_norm(h1[:, t, :], h_sb[:, t, :], 0, 1, "a",
                                   fast=(p == 2))
                        # h1T (f16) via PE transpose (f32 in PSUM, copy
                        # converts)
                        for kk in range(KB):
                            ptr = p1ps.tile([P, 512], f32, tag="mm", bufs=4,
                                            name="ptr3")
                            nc.tensor.transpose(ptr[:, 0:P],
                                                h1[:, t, ts(kk, P)], ident32)
                            nc.scalar.copy(out=h1T[:, kk, ts(t, P)],
                                           in_=ptr[:, 0:P])
                    for f in range(NF):
                        pg = p1ps.tile([P, 512], f32, tag="mm", bufs=4,
                                       name="pg")
                        for cc in range(KB // 2):
                            nc.tensor.matmul(
                                pg[:, 0:nt * P],
                                w1_sb[:, 2 * cc:2 * cc + 2, ts(f, P)],
                                h1T[:, 2 * cc:2 * cc + 2, :],
                                start=(cc == 0), stop=(cc == KB // 2 - 1),
                                perf_mode=mybir.MatmulPerfMode.DoubleRow)
                        nc.scalar.activation(
                            out=gT[:, f, :], in_=pg[:, 0:nt * P],
                            func=mybir.ActivationFunctionType.Relu,
                            bias=b1_sb[:, f:f + 1], scale=1.0 / 64.0)
                    for t in range(nt):
                        po = p1ps.tile([P, 512], f32, tag="mm", bufs=4,
                                       name="po")
                        for cc in range(NF // 2):
                            nc.tensor.matmul(
                                po, gT[:, 2 * cc:2 * cc + 2, ts(t, P)],
                                w2_sb[:, 2 * cc:2 * cc + 2, :],
                                start=(cc == 0), stop=(cc == NF // 2 - 1),
                                perf_mode=mybir.MatmulPerfMode.DoubleRow)
                        o2 = p1s.tile([P, D], f32, tag="o2", bufs=2)
                        nc.vector.scalar_tensor_tensor(
                            out=o2, in0=po, scalar=1.0 / 64.0,
                            in1=lnp_sb[:, 4, :],
                            op0=mybir.AluOpType.mult,
                            op1=mybir.AluOpType.add)
                        last = (p == 2)
                        eng2 = nc.vector if last else nc.gpsimd
                        eng2.tensor_tensor(out=o2, in0=o2,
                                           in1=h1[:, t, :],
                                           op=mybir.AluOpType.add)
                        yt = p1s.tile([P, D], f32, tag="yt", bufs=2)
                        layer_norm(yt, o2, 2, 3, "b", fast=last)
                        tg = toff + t
                        nc.sync.dma_start(out=y[P * tg:P * (tg + 1), :],
                                          in_=yt)

                # software-pipelined schedule: each (J, hp) strip/gather
                # stage is emitted interleaved with the processing of a
                # stage two slots earlier, so gather DMA latency and the
                # PSUM->SBUF copy drain hide under score processing.
                # The first AllGather (ctx rows 0-1023) fires after the
                # J=1 stages; its FFN half is emitted after stage (3,0) so
                # it fills PE while the last attention stage's feeder
                # chain drains.
                if 1 in phases:
                    if 3 in phases:
                        pid = nc.sync.partition_id()
                        rsA = nc.sync.snap((pid % 4) * 256)
                        rsBC = nc.sync.snap((pid % 4) * P)
                    # ascending J: small stages first, and stage (J, *)
                    # needs only the first J+1 projection chunks, so the
                    # early strips/gathers hide under phase-0 PE work
                    stages = [(J, hp) for J in range(4)
                              for hp in range(2)]
                    for _ in stage_strips(*stages[0]):
                        pass
                    for _ in stage_strips(*stages[1]):
                        pass
                    for i, (J, hp) in enumerate(stages):
                        sg = (stage_strips(*stages[i + 2])
                              if i + 2 < len(stages) else None)
                        for _ in stage_proc(J, hp):
                            if sg is not None:
                                next(sg, None)
                        if sg is not None:
                            for _ in sg:
                                pass
                        if 3 in phases and i == 3:
                            # dummy Sqrt: pull the P3 act-table load into
                            # the attention phase where Act has slack
                            nc.scalar.activation(
                                out=eps_sb, in_=eps_sb,
                                func=mybir.ActivationFunctionType.Sqrt)
                            nc.vector.memset(eps_sb, EPS)
                            collective_part(0)
                        if 3 in phases and i == 5:
                            collective_part(1)
                            phase3_part(0, rsA)
                        if 3 in phases and i == 6:
                            phase3_part(1, rsBC)
                    if 3 in phases:
                        collective_part(2)
                        phase3_part(2, rsBC)

    nc.finalize()
    return nc


def _prep_inputs(x, Wq, bq, Wk, bk, Wv, bv, Er, W1, b1, W2, b2, g1, be1, g2, be2):
    x = np.asarray(x, np.float32)
    ert_pad = np.zeros((DH, ERT_W), np.float16)
    ert_pad[:, :S] = np.asarray(Er, np.float32).T.astype(np.float16)
    in_maps = []
    for c in range(NCORES):
        b = c // 4
        g = c % 4
        cols = slice(P * g, P * (g + 1))
        # FFN rows: this core's share of the three gathered row-parts
        rowsel = np.r_[256 * g:256 * g + 256,
                       1024 + 128 * g:1024 + 128 * g + 128,
                       1536 + 128 * g:1536 + 128 * g + 128]
        m = {
            "xT": np.ascontiguousarray(x[b].T).astype(np.float16),
            "wq": np.ascontiguousarray(
                np.asarray(Wq, np.float32)[:, cols] / 8.0).astype(np.float16),
            "wk": np.ascontiguousarray(
                np.asarray(Wk, np.float32)[:, cols]).astype(np.float16),
            "wv": np.ascontiguousarray(
                np.asarray(Wv, np.float32)[:, cols]).astype(np.float16),
            "bqkv": np.stack([np.asarray(bq, np.float32)[cols] / 8.0,
                              np.asarray(bk, np.float32)[cols],
                              np.asarray(bv, np.float32)[cols]]),
            "ert": ert_pad,
            "xres": np.ascontiguousarray(x[b, rowsel]).astype(np.float16),
            "w1": np.ascontiguousarray(np.asarray(W1, np.float32) * 64.0
                                       ).astype(mybir.dt.np(f8)),
            "w2": np.ascontiguousarray(np.asarray(W2, np.float32) * 64.0
                                       ).astype(mybir.dt.np(f8)),
            "b1": np.ascontiguousarray(np.asarray(b1, np.float32).reshape(NF, P).T),
            "lnp": np.stack([np.asarray(g1, np.float32),
                             np.asarray(be1, np.float32),
                             np.asarray(g2, np.float32),
                             np.asarray(be2, np.float32),
                             np.asarray(b2, np.float32)]).astype(np.float16),
        }
        in_maps.append(m)
    return in_maps


def _get_runner():
    """Build the SPMD jax executable once and cache it."""
    if "runner" in _COMPILED:
        return _COMPILED["runner"]
    import jax
    from jax.experimental.shard_map import shard_map
    from jax.sharding import Mesh, PartitionSpec
    import concourse.mybir as _mybir
    from concourse import bass2jax as b2j

    nc = build_nc()
    b2j.install_neuronx_cc_hook()
    partition_name = (nc.partition_id_tensor.name
                      if nc.partition_id_tensor else None)
    in_names, out_names, out_avals, zero_shapes = [], [], [], []
    for alloc in nc.m.functions[0].allocations:
        if not isinstance(alloc, _mybir.MemoryLocationSet):
            continue
        name = alloc.memorylocations[0].name
        if alloc.kind == "ExternalInput":
            if name != partition_name:
                in_names.append(name)
        elif alloc.kind == "ExternalOutput":
            out_names.append(name)
            shape = tuple(alloc.tensor_shape)
            dtype = _mybir.dt.np(alloc.dtype)
            out_avals.append(jax.core.ShapedArray(shape, dtype))
            zero_shapes.append((shape, dtype))
    n_params = len(in_names)
    n_outs = len(out_avals)
    all_names = in_names + out_names
    if partition_name is not None:
        all_names = all_names + [partition_name]
    donate = tuple(range(n_params, n_params + n_outs))

    def _body(*args):
        operands = list(args)
        if partition_name is not None:
            operands.append(b2j.partition_id_tensor())
        return tuple(b2j._bass_exec_p.bind(
            *operands, out_avals=tuple(out_avals), in_names=tuple(all_names),
            out_names=tuple(out_names), lowering_input_output_aliases=(),
            sim_require_finite=True, sim_require_nnan=True, nc=nc))

    devices = jax.devices()[:NCORES]
    mesh = Mesh(np.asarray(devices), ("core",))
    in_specs = (PartitionSpec("core"),) * (n_params + n_outs)
    out_specs = (PartitionSpec("core"),) * len(out_names)
    sharded = jax.jit(shard_map(_body, mesh=mesh, in_specs=in_specs,
                                out_specs=out_specs, check_rep=False),
                      donate_argnums=donate, keep_unused=True)

    def runner(in_maps):
        concat_in = [np.concatenate([np.asarray(in_maps[c][n])
                                     for c in range(NCORES)], axis=0)
                     for n in in_names]
        concat_zeros = [np.zeros((NCORES * s[0], *s[1:]), d)
                        for s, d in zero_shapes]
        out_arrs = sharded(*concat_in, *concat_zeros)
        return [{name: np.asarray(out_arrs[i]).reshape(
                    NCORES, *out_avals[i].shape)[c]
                 for i, name in enumerate(out_names)}
                for c in range(NCORES)]

    def bench(in_maps, iters=20):
        """Device-resident execution; returns (sync_times, async_batch_avg).

        sync_times: per-call wall with block_until_ready (includes RPC).
        async_batch_avg: N calls queued without blocking, then one sync —
        per-call time when dispatch pipelines with execution.
        """
        import time as _t
        from jax.sharding import NamedSharding
        sh = NamedSharding(mesh, PartitionSpec("core"))
        concat_in = [jax.device_put(
            np.concatenate([np.asarray(in_maps[c][n])
                            for c in range(NCORES)], axis=0), sh)
            for n in in_names]
        zero_sets = []
        for _ in range(iters):
            zs = [jax.device_put(np.zeros((NCORES * s[0], *s[1:]), d), sh)
                  for s, d in zero_shapes]
            for z in zs:
                z.block_until_ready()
            zero_sets.append(zs)
        times = []
        for i in range(4):
            t0 = _t.time()
            outs = sharded(*concat_in, *zero_sets[i])
            for o in outs:
                o.block_until_ready()
            times.append(_t.time() - t0)
        t0 = _t.time()
        all_outs = []
        for i in range(4, iters):
            all_outs.append(sharded(*concat_in, *zero_sets[i]))
        for outs in all_outs:
            for o in outs:
                o.block_until_ready()
        async_avg = (_t.time() - t0) / (iters - 4)
        return times, async_avg

    _COMPILED["runner"] = runner
    _COMPILED["bench"] = bench
    return runner


def get_bench():
    _get_runner()
    return _COMPILED["bench"]


def kernel(**inputs):
    in_maps = _prep_inputs(**inputs)
    results = _get_runner()(in_maps)
    out = np.empty((B, S, D), np.float32)
    for c in range(NCORES):
        b, g = c // 4, c % 4
        yc = results[c]["y"]
        out[b, 256 * g:256 * (g + 1), :] = yc[:256]
        out[b, 1024 + 128 * g:1024 + 128 * (g + 1), :] = yc[256:384]
        out[b, 1536 + 128 * g:1536 + 128 * (g + 1), :] = yc[384:]
    return out
